# revision 17
# baseline (speedup 1.0000x reference)
"""Causal attention kernel for Trainium2 (8 NeuronCores, Bass/Tile).

Problem: B=4, S=2048, D=1024 fp32.
  qkv = x @ [Wq|Wk|Wv]; S = mask(q@k.T); P = softmax(S/sqrt(D)); ctx = P@v
  returns (context [4,2048,1024], attn_scores [4,2048,2048])

Sharding: core = (batch b = core//2, parity p = core%2). Each core handles
8 query blocks of 128 rows of its batch, chosen causally balanced:
  parity 0 -> q-blocks [0,15,2,13,4,11,6,9]
  parity 1 -> q-blocks [1,14,3,12,5,10,7,8]
Both parities run the same SPMD program with padded key extents
E_SCHED = [2,16,4,14,6,12,8,10] (in 128-blocks); the true causal boundary
(which is parity-dependent) is applied via a host-provided additive mask
over the last 256 key columns of each tile.

On-device phases (single NEFF):
  A1: K^T = (Wk.T @ x.T) and Q^T (for the core's q rows) -> DRAM bounce
  A2: V = x @ Wv -> resident in SBUF
  B:  per q-tile: S = Q^T.T @ K^T (psum), mask, row-max, exp((S-max)/32),
      P -> DMA out (normalized), P^T via PE transpose, ctx = P^T.T @ V,
      scale by 1/rowsum -> DMA out.

All matmuls fp32 (logits need fp32 accuracy: softmax here is near-one-hot
with min top-2 logit gap ~0.57).
"""

import os
import sys
from contextlib import ExitStack

import numpy as np

import concourse.bacc as bacc
import concourse.bass as bass
import concourse.mybir as mybir
import concourse.tile as tile
from concourse.bass_utils import run_bass_kernel_spmd
from concourse.masks import make_identity

B, S, D = 4, 2048, 1024
PB = 128
NT = 8  # q-tiles per core
E_SCHED = [2, 16, 4, 14, 6, 12, 8, 10]  # padded key extents per tile (128-blocks)
QBLOCKS = [
    [0, 15, 2, 13, 4, 11, 6, 9],  # parity 0
    [1, 14, 3, 12, 5, 10, 7, 8],  # parity 1
]
SCALE = 1.0 / 32.0  # 1/sqrt(1024)
NEG = -1.0e30
F32 = mybir.dt.float32
F32R = mybir.dt.float32r  # reduced-precision matmul fmt, 4x PE rate at N>=256
KD = D // PB  # 8 contraction blocks


def _chunks(width, step=512):
    return [(c, min(step, width - c)) for c in range(0, width, step)]


def _build_program():
    nc = bacc.Bacc(trn_type="TRN2", target_bir_lowering=False, debug=False, num_swdge_queues=4)

    xT = nc.dram_tensor("xT", [D, S], F32, kind="ExternalInput").ap()
    xTq = nc.dram_tensor("xTq", [D, NT * PB], F32, kind="ExternalInput").ap()
    Wq = nc.dram_tensor("Wq", [D, D], F32, kind="ExternalInput").ap()
    Wk = nc.dram_tensor("Wk", [D, D], F32, kind="ExternalInput").ap()
    Wv = nc.dram_tensor("Wv", [D, D], F32, kind="ExternalInput").ap()
    masks = nc.dram_tensor("masks", [NT, PB, 256], F32, kind="ExternalInput").ap()
    s_out = nc.dram_tensor("s_out", [NT, PB, S], F32, kind="ExternalOutput").ap()
    ctx_out = nc.dram_tensor("ctx_out", [NT, PB, D], F32, kind="ExternalOutput").ap()
    kT_d = nc.dram_tensor("kT_d", [D, S], F32).ap()
    qT_d = nc.dram_tensor("qT_d", [D, NT * PB], F32).ap()

    NQ = NT * PB  # 1024 query rows per core

    with tile.TileContext(nc) as tc, ExitStack() as top:
        const = top.enter_context(tc.tile_pool(name="const", bufs=1))
        vpool = top.enter_context(tc.tile_pool(name="vres", bufs=1))

        ident = const.tile([PB, PB], F32)
        make_identity(nc, ident[:, :])
        mask_sb = const.tile([PB, NT * 256], F32)
        nc.sync.dma_start(
            out=mask_sb[:, :].rearrange("p (t c) -> p t c", t=NT),
            in_=masks.rearrange("t p c -> p t c"),
        )
        v = vpool.tile([PB, (S // PB) * D], F32R)  # V resident (f32r): block s at cols s*D

        # ---- phase A: projections ----
        with tc.tile_pool(name="xtp", bufs=1) as xtp:
            xt = xtp.tile([PB, KD * S], F32)  # x.T resident: block k at cols k*S
            for k in range(KD):
                nc.sync.dma_start(
                    out=xt[:, k * S : (k + 1) * S], in_=xT[k * PB : (k + 1) * PB, :]
                )

            # A1: K^T and Q^T -> DRAM
            with (
                tc.tile_pool(name="xtqp", bufs=1) as xtqp,
                tc.tile_pool(name="a1w", bufs=2) as a1w,
                tc.tile_pool(name="bnc", bufs=4) as bnc,
                tc.tile_pool(name="psA", bufs=6, space="PSUM") as psA,
            ):
                xtq = xtqp.tile([PB, KD * NQ], F32)
                for k in range(KD):
                    nc.sync.dma_start(
                        out=xtq[:, k * NQ : (k + 1) * NQ],
                        in_=xTq[k * PB : (k + 1) * PB, :],
                    )
                for m in range(KD):
                    wkt = a1w.tile([PB, KD * PB], F32, tag="wk", name="wkt")
                    nc.sync.dma_start(
                        out=wkt[:, :].rearrange("p (k m) -> p k m", k=KD),
                        in_=Wk.rearrange("(k p) m -> p k m", p=PB)[
                            :, :, m * PB : (m + 1) * PB
                        ],
                    )
                    # k-outer so the first matmuls only need xt block 0
                    # (hides the initial xt DMA under compute)
                    kchunks = _chunks(S)
                    pss = [
                        psA.tile([PB, 512], F32, tag="psA", name=f"psK{ci}")
                        for ci in range(len(kchunks))
                    ]
                    for k in range(KD):
                        for ci, (c, w) in enumerate(kchunks):
                            nc.tensor.matmul(
                                pss[ci][:, :w],
                                wkt[:, k * PB : (k + 1) * PB],
                                xt[:, k * S + c : k * S + c + w],
                                start=(k == 0),
                                stop=(k == KD - 1),
                            )
                    for ci, (c, w) in enumerate(kchunks):
                        ob = bnc.tile([PB, 512], F32, tag="ob")
                        nc.scalar.copy(ob[:, :w], pss[ci][:, :w])
                        nc.sync.dma_start(
                            out=kT_d[m * PB : (m + 1) * PB, c : c + w], in_=ob[:, :w]
                        )
                    wqt = a1w.tile([PB, KD * PB], F32, tag="wq", name="wqt")
                    nc.sync.dma_start(
                        out=wqt[:, :].rearrange("p (k m) -> p k m", k=KD),
                        in_=Wq.rearrange("(k p) m -> p k m", p=PB)[
                            :, :, m * PB : (m + 1) * PB
                        ],
                    )
                    qchunks = _chunks(NQ)
                    psq = [
                        psA.tile([PB, 512], F32, tag="psA", name=f"psQ{ci}")
                        for ci in range(len(qchunks))
                    ]
                    for k in range(KD):
                        for ci, (c, w) in enumerate(qchunks):
                            nc.tensor.matmul(
                                psq[ci][:, :w],
                                wqt[:, k * PB : (k + 1) * PB],
                                xtq[:, k * NQ + c : k * NQ + c + w],
                                start=(k == 0),
                                stop=(k == KD - 1),
                            )
                    for ci, (c, w) in enumerate(qchunks):
                        ob = bnc.tile([PB, 512], F32, tag="ob")
                        nc.scalar.copy(ob[:, :w], psq[ci][:, :w])
                        nc.sync.dma_start(
                            out=qT_d[m * PB : (m + 1) * PB, c : c + w], in_=ob[:, :w]
                        )

            # A2: V = x @ Wv in float32r (value path tolerates reduced
            # precision; logit path stays fp32), V resident as f32r
            with (
                tc.tile_pool(name="wvp", bufs=1) as wvp,
                tc.tile_pool(name="wvstg", bufs=2) as wvstg,
                tc.tile_pool(name="xtrp", bufs=12) as xtrp,
                tc.tile_pool(name="psV", bufs=4, space="PSUM") as psV,
            ):
                wv = wvp.tile([PB, KD * D], F32R)  # Wv block k at cols k*D
                for k in range(KD):
                    stg = wvstg.tile([PB, D], F32, tag="stg", name="stg")
                    nc.sync.dma_start(
                        out=stg[:, :], in_=Wv[k * PB : (k + 1) * PB, :]
                    )
                    nc.vector.tensor_copy(wv[:, k * D : (k + 1) * D], stg[:, :])
                for sblk in range(S // PB):
                    xtr = []
                    for k in range(KD):
                        xr = xtrp.tile([PB, PB], F32R, tag="xtr", name="xr")
                        nc.vector.tensor_copy(
                            xr[:, :],
                            xt[:, k * S + sblk * PB : k * S + (sblk + 1) * PB],
                        )
                        xtr.append(xr)
                    for h in range(2):
                        ps = psV.tile([PB, 512], F32, tag="psV")
                        for k in range(KD):
                            nc.tensor.matmul(
                                ps[:, :],
                                xtr[k][:, :],
                                wv[:, k * D + h * 512 : k * D + (h + 1) * 512],
                                start=(k == 0),
                                stop=(k == KD - 1),
                            )
                        nc.scalar.copy(
                            v[:, sblk * D + h * 512 : sblk * D + (h + 1) * 512],
                            ps[:, :],
                        )

        # ---- phase B: attention ----
        with (
            tc.tile_pool(name="qtp", bufs=2) as qtp,
            tc.tile_pool(name="ktcp", bufs=24) as ktcp,
            tc.tile_pool(name="pp", bufs=2) as pp,
            tc.tile_pool(name="pnp", bufs=2) as pnp,
            tc.tile_pool(name="ptp", bufs=4) as ptp,
            tc.tile_pool(name="cbp", bufs=4) as cbp,
            tc.tile_pool(name="stat", bufs=3) as stat,
            tc.tile_pool(name="psS", bufs=2, space="PSUM") as psS,
            tc.tile_pool(name="psT", bufs=2, space="PSUM") as psT,
            tc.tile_pool(name="psC", bufs=2, space="PSUM") as psC,
        ):
            HALF = 1024
            for t in range(NT):
                E = E_SCHED[t]
                W = E * PB
                qt = qtp.tile([PB, KD * PB], F32, tag="qt")
                nc.sync.dma_start(
                    out=qt[:, :].rearrange("p (k q) -> p k q", k=KD),
                    in_=qT_d.rearrange("(k p) q -> p k q", p=PB)[
                        :, :, t * PB : (t + 1) * PB
                    ],
                )
                # S psum in half-tiles of 1024 so tile t+1 can reuse slots
                # as soon as tile t's exp drains them
                nhalf = (W + HALF - 1) // HALF
                pshs = [
                    psS.tile([PB, HALF], F32, tag="ps", name=f"psh{h}")
                    for h in range(nhalf)
                ]
                for (c, w) in _chunks(W):
                    hi, lo = c // HALF, c % HALF
                    for k in range(KD):
                        ktc = ktcp.tile([PB, 512], F32, tag="ktc")
                        nc.sync.dma_start(
                            out=ktc[:, :w], in_=kT_d[k * PB : (k + 1) * PB, c : c + w]
                        )
                        nc.tensor.matmul(
                            pshs[hi][:, lo : lo + w],
                            qt[:, k * PB : (k + 1) * PB],
                            ktc[:, :w],
                            start=(k == 0),
                            stop=(k == KD - 1),
                        )
                # causal mask on the last two key blocks (256-aligned, never
                # spans a half boundary)
                mhi, mlo = (W - 256) // HALF, (W - 256) % HALF
                nc.vector.tensor_add(
                    pshs[mhi][:, mlo : mlo + 256],
                    pshs[mhi][:, mlo : mlo + 256],
                    mask_sb[:, t * 256 : (t + 1) * 256],
                )
                hws = [min(HALF, W - h * HALF) for h in range(nhalf)]
                if nhalf == 1:
                    rowmax = stat.tile([PB, 1], F32, tag="rmax")
                    nc.vector.reduce_max(
                        rowmax[:, :], pshs[0][:, : hws[0]], axis=mybir.AxisListType.X
                    )
                else:
                    hmax = stat.tile([PB, 2], F32, tag="hmax")
                    for h in range(nhalf):
                        nc.vector.reduce_max(
                            hmax[:, h : h + 1],
                            pshs[h][:, : hws[h]],
                            axis=mybir.AxisListType.X,
                        )
                    rowmax = stat.tile([PB, 1], F32, tag="rmax")
                    nc.vector.reduce_max(
                        rowmax[:, :], hmax[:, :], axis=mybir.AxisListType.X
                    )
                ebias = stat.tile([PB, 1], F32, tag="ebias")
                nc.vector.tensor_scalar_mul(ebias[:, :], rowmax[:, :], -SCALE)
                p_t = pp.tile([PB, S], F32, tag="p")
                for h in range(nhalf):
                    nc.scalar.activation(
                        p_t[:, h * HALF : h * HALF + hws[h]],
                        pshs[h][:, : hws[h]],
                        mybir.ActivationFunctionType.Exp,
                        bias=ebias[:, :],
                        scale=SCALE,
                    )
                rowsum = stat.tile([PB, 1], F32, tag="rsum")
                nc.vector.reduce_sum(rowsum[:, :], p_t[:, :W], axis=mybir.AxisListType.X)
                recip = stat.tile([PB, 1], F32, tag="recip")
                nc.vector.reciprocal(recip[:, :], rowsum[:, :])
                pn = pnp.tile([PB, S], F32, tag="pn")
                nc.vector.tensor_scalar_mul(pn[:, :W], p_t[:, :W], recip[:, :])
                nc.sync.dma_start(out=s_out[t, :, 0:W], in_=pn[:, :W])
                # ctx = P^T.T @ V, scaled by recip
                pc = [
                    psC.tile([PB, 512], F32, tag="pc", name=f"pc{d}")
                    for d in range(2)
                ]
                for e in range(E):
                    pst = psT.tile([PB, PB], F32, tag="pst")
                    nc.tensor.transpose(
                        pst[:, :], p_t[:, e * PB : (e + 1) * PB], ident[:, :]
                    )
                    pte = ptp.tile([PB, PB], F32R, tag="pte")
                    nc.vector.tensor_copy(pte[:, :], pst[:, :])
                    for d in range(2):
                        nc.tensor.matmul(
                            pc[d][:, :],
                            pte[:, :],
                            v[:, e * D + d * 512 : e * D + (d + 1) * 512],
                            start=(e == 0),
                            stop=(e == E - 1),
                        )
                for d in range(2):
                    cb = cbp.tile([PB, 512], F32, tag="cb")
                    nc.vector.tensor_scalar_mul(cb[:, :], pc[d][:, :], recip[:, :])
                    nc.sync.dma_start(
                        out=ctx_out[t, :, d * 512 : (d + 1) * 512], in_=cb[:, :]
                    )
    nc.compile()
    return nc


_CACHE = {}


def _program():
    if "nc" not in _CACHE:
        _CACHE["nc"] = _build_program()
    return _CACHE["nc"]


def _tri_mask():
    r = np.arange(PB)
    return np.where(r[None, :] <= r[:, None], 0.0, NEG).astype(np.float32)


def _masks_for_parity(p):
    tri = _tri_mask()
    m = np.zeros((NT, PB, 256), np.float32)
    for t in range(NT):
        E = E_SCHED[t]
        e = QBLOCKS[p][t] + 1  # true causal extent in blocks
        if e == E:
            m[t, :, 128:] = tri
        elif e == E - 1:
            m[t, :, :128] = tri
            m[t, :, 128:] = NEG
        else:
            raise AssertionError("bad schedule")
    return m


def _in_maps(x, Wq, Wk, Wv):
    maps = []
    pmasks = [_masks_for_parity(0), _masks_for_parity(1)]
    for core in range(8):
        b, p = divmod(core, 2)
        xTb = np.ascontiguousarray(x[b].T)
        qrows = np.concatenate(
            [np.arange(g * PB, (g + 1) * PB) for g in QBLOCKS[p]]
        )
        xTqb = np.ascontiguousarray(xTb[:, qrows])
        maps.append(
            {
                "xT": xTb,
                "xTq": xTqb,
                "Wq": Wq,
                "Wk": Wk,
                "Wv": Wv,
                "masks": pmasks[p],
            }
        )
    return maps


def _assemble(results):
    scores = np.zeros((B, S, S), np.float32)
    ctx = np.empty((B, S, D), np.float32)
    for core in range(8):
        b, p = divmod(core, 2)
        so = results[core]["s_out"]
        co = results[core]["ctx_out"]
        for t, g in enumerate(QBLOCKS[p]):
            scores[b, g * PB : (g + 1) * PB, :] = so[t]
            ctx[b, g * PB : (g + 1) * PB, :] = co[t]
    return ctx, scores


def run_on_cores(x, W_query, W_key, W_value, **spmd_kwargs):
    """Run the 8-core SPMD program; returns (BassKernelResults, (ctx, scores))."""
    x = np.ascontiguousarray(np.asarray(x, dtype=np.float32))
    Wq = np.ascontiguousarray(np.asarray(W_query, dtype=np.float32))
    Wk = np.ascontiguousarray(np.asarray(W_key, dtype=np.float32))
    Wv = np.ascontiguousarray(np.asarray(W_value, dtype=np.float32))
    nc = _program()
    res = run_bass_kernel_spmd(nc, _in_maps(x, Wq, Wk, Wv), list(range(8)), **spmd_kwargs)
    return res, _assemble(res.results)


def kernel(x, W_query, W_key, W_value):
    _, out = run_on_cores(x, W_query, W_key, W_value)
    return out


# revision 19
# speedup vs baseline: 1.1695x; 1.1695x over previous
"""Causal attention kernel for Trainium2 (8 NeuronCores, Bass/Tile).

Problem: B=4, S=2048, D=1024 fp32.
  qkv = x @ [Wq|Wk|Wv]; S = mask(q@k.T); P = softmax(S/sqrt(D)); ctx = P@v
  returns (context [4,2048,1024], attn_scores [4,2048,2048])

Sharding: core = (batch b = core//2, parity p = core%2). Each core handles
8 query blocks of 128 rows of its batch, chosen causally balanced:
  parity 0 -> q-blocks [0,15,2,13,4,11,6,9]
  parity 1 -> q-blocks [1,14,3,12,5,10,7,8]
Both parities run the same SPMD program with padded key extents
E_SCHED = [2,16,4,14,6,12,8,10] (in 128-blocks); the true causal boundary
(which is parity-dependent) is applied via a host-provided additive mask
over the last 256 key columns of each tile.

On-device phases (single NEFF):
  A1: K^T = (Wk.T @ x.T) and Q^T (for the core's q rows) -> DRAM bounce
  A2: V = x @ Wv -> resident in SBUF
  B:  per q-tile: S = Q^T.T @ K^T (psum), mask, row-max, exp((S-max)/32),
      P -> DMA out (normalized), P^T via PE transpose, ctx = P^T.T @ V,
      scale by 1/rowsum -> DMA out.

All matmuls fp32 (logits need fp32 accuracy: softmax here is near-one-hot
with min top-2 logit gap ~0.57).
"""

import os
import sys
from contextlib import ExitStack

import numpy as np

import concourse.bacc as bacc
import concourse.bass as bass
import concourse.mybir as mybir
import concourse.tile as tile
from concourse.bass_utils import run_bass_kernel_spmd
from concourse.masks import make_identity

B, S, D = 4, 2048, 1024
PB = 128
NT = 8  # q-tiles per core
E_SCHED = [2, 16, 4, 14, 6, 12, 8, 10]  # padded key extents per tile (128-blocks)
QBLOCKS = [
    [0, 15, 2, 13, 4, 11, 6, 9],  # parity 0
    [1, 14, 3, 12, 5, 10, 7, 8],  # parity 1
]
SCALE = 1.0 / 32.0  # 1/sqrt(1024)
NEG = -1.0e30
F32 = mybir.dt.float32
F32R = mybir.dt.float32r  # reduced-precision matmul fmt, 4x PE rate at N>=256
KD = D // PB  # 8 contraction blocks


def _chunks(width, step=512):
    return [(c, min(step, width - c)) for c in range(0, width, step)]


def _build_program():
    nc = bacc.Bacc(trn_type="TRN2", target_bir_lowering=False, debug=False, num_swdge_queues=4)

    xT = nc.dram_tensor("xT", [D, S], F32, kind="ExternalInput").ap()
    xTq = nc.dram_tensor("xTq", [D, NT * PB], F32, kind="ExternalInput").ap()
    Wq = nc.dram_tensor("Wq", [D, D], F32, kind="ExternalInput").ap()
    Wk = nc.dram_tensor("Wk", [D, D], F32, kind="ExternalInput").ap()
    Wv = nc.dram_tensor("Wv", [D, D], F32, kind="ExternalInput").ap()
    masks = nc.dram_tensor("masks", [NT, PB, 256], F32, kind="ExternalInput").ap()
    s_out = nc.dram_tensor("s_out", [NT, PB, S], F32, kind="ExternalOutput").ap()
    ctx_out = nc.dram_tensor("ctx_out", [NT, PB, D], F32, kind="ExternalOutput").ap()
    kT_d = nc.dram_tensor("kT_d", [D, S], F32).ap()
    qT_d = nc.dram_tensor("qT_d", [D, NT * PB], F32).ap()

    NQ = NT * PB  # 1024 query rows per core

    with tile.TileContext(nc) as tc, ExitStack() as top:
        const = top.enter_context(tc.tile_pool(name="const", bufs=1))
        vpool = top.enter_context(tc.tile_pool(name="vres", bufs=1))

        ident = const.tile([PB, PB], F32)
        make_identity(nc, ident[:, :])
        mask_sb = const.tile([PB, NT * 256], F32)
        v = vpool.tile([PB, (S // PB) * D], F32R)  # V resident (f32r): block s at cols s*D

        # ---- phase A: projections ----
        with tc.tile_pool(name="xtp", bufs=1) as xtp:
            xt = xtp.tile([PB, KD * S], F32)  # x.T resident: block k at cols k*S

            # A1: Q^T first (needs only xtq, 4MB) so PE starts early while
            # the bigger xt load streams in; then K^T.
            with (
                tc.tile_pool(name="xtqp", bufs=1) as xtqp,
                tc.tile_pool(name="a1w", bufs=2) as a1w,
                tc.tile_pool(name="bnc", bufs=4) as bnc,
                tc.tile_pool(name="psA", bufs=6, space="PSUM") as psA,
            ):
                # first Q weight block loads ahead of the bulk x loads so
                # the PE can start within a few us
                wqt0 = a1w.tile([PB, KD * PB], F32, tag="wq", name="wqt0")
                nc.sync.dma_start(
                    out=wqt0[:, :].rearrange("p (k m) -> p k m", k=KD),
                    in_=Wq.rearrange("(k p) m -> p k m", p=PB)[:, :, 0:PB],
                )
                xtq = xtqp.tile([PB, KD * NQ], F32)
                for k in range(KD):
                    nc.sync.dma_start(
                        out=xtq[:, k * NQ : (k + 1) * NQ],
                        in_=xTq[k * PB : (k + 1) * PB, :],
                    )
                for k in range(KD):
                    nc.sync.dma_start(
                        out=xt[:, k * S : (k + 1) * S], in_=xT[k * PB : (k + 1) * PB, :]
                    )
                nc.sync.dma_start(
                    out=mask_sb[:, :].rearrange("p (t c) -> p t c", t=NT),
                    in_=masks.rearrange("t p c -> p t c"),
                )
                for m in range(KD):
                    if m == 0:
                        wqt = wqt0
                    else:
                        wqt = a1w.tile([PB, KD * PB], F32, tag="wq", name="wqt")
                        nc.sync.dma_start(
                            out=wqt[:, :].rearrange("p (k m) -> p k m", k=KD),
                            in_=Wq.rearrange("(k p) m -> p k m", p=PB)[
                                :, :, m * PB : (m + 1) * PB
                            ],
                        )
                    qchunks = _chunks(NQ)
                    psq = [
                        psA.tile([PB, 512], F32, tag="psA", name=f"psQ{ci}")
                        for ci in range(len(qchunks))
                    ]
                    for k in range(KD):
                        for ci, (c, w) in enumerate(qchunks):
                            nc.tensor.matmul(
                                psq[ci][:, :w],
                                wqt[:, k * PB : (k + 1) * PB],
                                xtq[:, k * NQ + c : k * NQ + c + w],
                                start=(k == 0),
                                stop=(k == KD - 1),
                            )
                    for ci, (c, w) in enumerate(qchunks):
                        ob = bnc.tile([PB, 512], F32, tag="ob")
                        nc.scalar.copy(ob[:, :w], psq[ci][:, :w])
                        nc.sync.dma_start(
                            out=qT_d[m * PB : (m + 1) * PB, c : c + w], in_=ob[:, :w]
                        )
                for m in range(KD):
                    wkt = a1w.tile([PB, KD * PB], F32, tag="wk", name="wkt")
                    nc.sync.dma_start(
                        out=wkt[:, :].rearrange("p (k m) -> p k m", k=KD),
                        in_=Wk.rearrange("(k p) m -> p k m", p=PB)[
                            :, :, m * PB : (m + 1) * PB
                        ],
                    )
                    # k-outer so the first matmuls only need xt block 0
                    # (hides the initial xt DMA under compute)
                    kchunks = _chunks(S)
                    pss = [
                        psA.tile([PB, 512], F32, tag="psA", name=f"psK{ci}")
                        for ci in range(len(kchunks))
                    ]
                    for k in range(KD):
                        for ci, (c, w) in enumerate(kchunks):
                            nc.tensor.matmul(
                                pss[ci][:, :w],
                                wkt[:, k * PB : (k + 1) * PB],
                                xt[:, k * S + c : k * S + c + w],
                                start=(k == 0),
                                stop=(k == KD - 1),
                            )
                    for ci, (c, w) in enumerate(kchunks):
                        ob = bnc.tile([PB, 512], F32, tag="ob")
                        nc.scalar.copy(ob[:, :w], pss[ci][:, :w])
                        nc.sync.dma_start(
                            out=kT_d[m * PB : (m + 1) * PB, c : c + w], in_=ob[:, :w]
                        )

            # A2: V = x @ Wv in float32r (value path tolerates reduced
            # precision; logit path stays fp32), V resident as f32r
            with (
                tc.tile_pool(name="wvp", bufs=1) as wvp,
                tc.tile_pool(name="wvstg", bufs=2) as wvstg,
                tc.tile_pool(name="xtrp", bufs=12) as xtrp,
                tc.tile_pool(name="psV", bufs=4, space="PSUM") as psV,
            ):
                wv = wvp.tile([PB, KD * D], F32R)  # Wv block k at cols k*D
                for k in range(KD):
                    stg = wvstg.tile([PB, D], F32, tag="stg", name="stg")
                    nc.sync.dma_start(
                        out=stg[:, :], in_=Wv[k * PB : (k + 1) * PB, :]
                    )
                    nc.vector.tensor_copy(wv[:, k * D : (k + 1) * D], stg[:, :])
                for sblk in range(S // PB):
                    xtr = []
                    for k in range(KD):
                        xr = xtrp.tile([PB, PB], F32R, tag="xtr", name="xr")
                        nc.vector.tensor_copy(
                            xr[:, :],
                            xt[:, k * S + sblk * PB : k * S + (sblk + 1) * PB],
                        )
                        xtr.append(xr)
                    for h in range(2):
                        ps = psV.tile([PB, 512], F32, tag="psV")
                        for k in range(KD):
                            nc.tensor.matmul(
                                ps[:, :],
                                xtr[k][:, :],
                                wv[:, k * D + h * 512 : k * D + (h + 1) * 512],
                                start=(k == 0),
                                stop=(k == KD - 1),
                            )
                        nc.scalar.copy(
                            v[:, sblk * D + h * 512 : sblk * D + (h + 1) * 512],
                            ps[:, :],
                        )

        # ---- phase B: attention ----
        with (
            tc.tile_pool(name="qtp", bufs=2) as qtp,
            tc.tile_pool(name="ktcp", bufs=9) as ktcp,
            tc.tile_pool(name="pp", bufs=2) as pp,
            tc.tile_pool(name="pnp", bufs=2) as pnp,
            tc.tile_pool(name="ptp", bufs=4) as ptp,
            tc.tile_pool(name="cbp", bufs=4) as cbp,
            tc.tile_pool(name="stat", bufs=3) as stat,
            tc.tile_pool(name="psS", bufs=2, space="PSUM") as psS,
            tc.tile_pool(name="psT", bufs=2, space="PSUM") as psT,
            tc.tile_pool(name="psC", bufs=2, space="PSUM") as psC,
        ):
            HALF = 1024
            for t in range(NT):
                E = E_SCHED[t]
                W = E * PB
                qt = qtp.tile([PB, KD * PB], F32, tag="qt")
                nc.sync.dma_start(
                    out=qt[:, :].rearrange("p (k q) -> p k q", k=KD),
                    in_=qT_d.rearrange("(k p) q -> p k q", p=PB)[
                        :, :, t * PB : (t + 1) * PB
                    ],
                )
                # S psum in half-tiles of 1024 so tile t+1 can reuse slots
                # as soon as tile t's exp drains them
                nhalf = (W + HALF - 1) // HALF
                pshs = [
                    psS.tile([PB, HALF], F32, tag="ps", name=f"psh{h}")
                    for h in range(nhalf)
                ]
                ktcs = []
                for k in range(KD):
                    ktc = ktcp.tile([PB, S], F32, tag="ktc", name="ktc")
                    eng = nc.sync if k % 2 == 0 else nc.scalar
                    eng.dma_start(
                        out=ktc[:, :W], in_=kT_d[k * PB : (k + 1) * PB, 0:W]
                    )
                    ktcs.append(ktc)
                for (c, w) in _chunks(W):
                    hi, lo = c // HALF, c % HALF
                    for k in range(KD):
                        nc.tensor.matmul(
                            pshs[hi][:, lo : lo + w],
                            qt[:, k * PB : (k + 1) * PB],
                            ktcs[k][:, c : c + w],
                            start=(k == 0),
                            stop=(k == KD - 1),
                        )
                # causal mask on the last two key blocks (256-aligned, never
                # spans a half boundary)
                mhi, mlo = (W - 256) // HALF, (W - 256) % HALF
                nc.vector.tensor_add(
                    pshs[mhi][:, mlo : mlo + 256],
                    pshs[mhi][:, mlo : mlo + 256],
                    mask_sb[:, t * 256 : (t + 1) * 256],
                )
                hws = [min(HALF, W - h * HALF) for h in range(nhalf)]
                if nhalf == 1:
                    rowmax = stat.tile([PB, 1], F32, tag="rmax")
                    nc.vector.reduce_max(
                        rowmax[:, :], pshs[0][:, : hws[0]], axis=mybir.AxisListType.X
                    )
                else:
                    hmax = stat.tile([PB, 2], F32, tag="hmax")
                    for h in range(nhalf):
                        nc.vector.reduce_max(
                            hmax[:, h : h + 1],
                            pshs[h][:, : hws[h]],
                            axis=mybir.AxisListType.X,
                        )
                    rowmax = stat.tile([PB, 1], F32, tag="rmax")
                    nc.vector.reduce_max(
                        rowmax[:, :], hmax[:, :], axis=mybir.AxisListType.X
                    )
                ebias = stat.tile([PB, 1], F32, tag="ebias")
                nc.vector.tensor_scalar_mul(ebias[:, :], rowmax[:, :], -SCALE)
                p_t = pp.tile([PB, S], F32, tag="p")
                for h in range(nhalf):
                    nc.scalar.activation(
                        p_t[:, h * HALF : h * HALF + hws[h]],
                        pshs[h][:, : hws[h]],
                        mybir.ActivationFunctionType.Exp,
                        bias=ebias[:, :],
                        scale=SCALE,
                    )
                rowsum = stat.tile([PB, 1], F32, tag="rsum")
                nc.vector.reduce_sum(rowsum[:, :], p_t[:, :W], axis=mybir.AxisListType.X)
                recip = stat.tile([PB, 1], F32, tag="recip")
                nc.vector.reciprocal(recip[:, :], rowsum[:, :])
                pn = pnp.tile([PB, S], F32, tag="pn")
                nc.vector.tensor_scalar_mul(pn[:, :W], p_t[:, :W], recip[:, :])
                nc.sync.dma_start(out=s_out[t, :, 0:W], in_=pn[:, :W])
                # ctx = P^T.T @ V, scaled by recip
                pc = [
                    psC.tile([PB, 512], F32, tag="pc", name=f"pc{d}")
                    for d in range(2)
                ]
                for e in range(E):
                    pst = psT.tile([PB, PB], F32, tag="pst")
                    nc.tensor.transpose(
                        pst[:, :], p_t[:, e * PB : (e + 1) * PB], ident[:, :]
                    )
                    pte = ptp.tile([PB, PB], F32R, tag="pte")
                    nc.vector.tensor_copy(pte[:, :], pst[:, :])
                    for d in range(2):
                        nc.tensor.matmul(
                            pc[d][:, :],
                            pte[:, :],
                            v[:, e * D + d * 512 : e * D + (d + 1) * 512],
                            start=(e == 0),
                            stop=(e == E - 1),
                        )
                for d in range(2):
                    cb = cbp.tile([PB, 512], F32, tag="cb")
                    nc.scalar.activation(
                        cb[:, :],
                        pc[d][:, :],
                        mybir.ActivationFunctionType.Copy,
                        scale=recip[:, :],
                    )
                    nc.sync.dma_start(
                        out=ctx_out[t, :, d * 512 : (d + 1) * 512], in_=cb[:, :]
                    )
    nc.compile()
    return nc


_CACHE = {}


def _program():
    if "nc" not in _CACHE:
        _CACHE["nc"] = _build_program()
    return _CACHE["nc"]


def _tri_mask():
    r = np.arange(PB)
    return np.where(r[None, :] <= r[:, None], 0.0, NEG).astype(np.float32)


def _masks_for_parity(p):
    tri = _tri_mask()
    m = np.zeros((NT, PB, 256), np.float32)
    for t in range(NT):
        E = E_SCHED[t]
        e = QBLOCKS[p][t] + 1  # true causal extent in blocks
        if e == E:
            m[t, :, 128:] = tri
        elif e == E - 1:
            m[t, :, :128] = tri
            m[t, :, 128:] = NEG
        else:
            raise AssertionError("bad schedule")
    return m


def _in_maps(x, Wq, Wk, Wv):
    maps = []
    pmasks = [_masks_for_parity(0), _masks_for_parity(1)]
    for core in range(8):
        b, p = divmod(core, 2)
        xTb = np.ascontiguousarray(x[b].T)
        qrows = np.concatenate(
            [np.arange(g * PB, (g + 1) * PB) for g in QBLOCKS[p]]
        )
        xTqb = np.ascontiguousarray(xTb[:, qrows])
        maps.append(
            {
                "xT": xTb,
                "xTq": xTqb,
                "Wq": Wq,
                "Wk": Wk,
                "Wv": Wv,
                "masks": pmasks[p],
            }
        )
    return maps


def _assemble(results):
    scores = np.zeros((B, S, S), np.float32)
    ctx = np.empty((B, S, D), np.float32)
    for core in range(8):
        b, p = divmod(core, 2)
        so = results[core]["s_out"]
        co = results[core]["ctx_out"]
        for t, g in enumerate(QBLOCKS[p]):
            scores[b, g * PB : (g + 1) * PB, :] = so[t]
            ctx[b, g * PB : (g + 1) * PB, :] = co[t]
    return ctx, scores


def run_on_cores(x, W_query, W_key, W_value, **spmd_kwargs):
    """Run the 8-core SPMD program; returns (BassKernelResults, (ctx, scores))."""
    x = np.ascontiguousarray(np.asarray(x, dtype=np.float32))
    Wq = np.ascontiguousarray(np.asarray(W_query, dtype=np.float32))
    Wk = np.ascontiguousarray(np.asarray(W_key, dtype=np.float32))
    Wv = np.ascontiguousarray(np.asarray(W_value, dtype=np.float32))
    nc = _program()
    res = run_bass_kernel_spmd(nc, _in_maps(x, Wq, Wk, Wv), list(range(8)), **spmd_kwargs)
    return res, _assemble(res.results)


def kernel(x, W_query, W_key, W_value):
    _, out = run_on_cores(x, W_query, W_key, W_value)
    return out


# revision 20
# speedup vs baseline: 1.1985x; 1.0248x over previous
"""Causal attention kernel for Trainium2 (8 NeuronCores, Bass/Tile).

Problem: B=4, S=2048, D=1024 fp32.
  qkv = x @ [Wq|Wk|Wv]; S = mask(q@k.T); P = softmax(S/sqrt(D)); ctx = P@v
  returns (context [4,2048,1024], attn_scores [4,2048,2048])

Sharding: core = (batch b = core//2, parity p = core%2). Each core handles
8 query blocks of 128 rows of its batch, chosen causally balanced:
  parity 0 -> q-blocks [0,15,2,13,4,11,6,9]
  parity 1 -> q-blocks [1,14,3,12,5,10,7,8]
Both parities run the same SPMD program with padded key extents
E_SCHED = [2,16,4,14,6,12,8,10] (in 128-blocks); the true causal boundary
(which is parity-dependent) is applied via a host-provided additive mask
over the last 256 key columns of each tile.

On-device phases (single NEFF):
  A1: K^T = (Wk.T @ x.T) and Q^T (for the core's q rows) -> DRAM bounce
  A2: V = x @ Wv -> resident in SBUF
  B:  per q-tile: S = Q^T.T @ K^T (psum), mask, row-max, exp((S-max)/32),
      P -> DMA out (normalized), P^T via PE transpose, ctx = P^T.T @ V,
      scale by 1/rowsum -> DMA out.

All matmuls fp32 (logits need fp32 accuracy: softmax here is near-one-hot
with min top-2 logit gap ~0.57).
"""

import os
import sys
from contextlib import ExitStack

import numpy as np

import concourse.bacc as bacc
import concourse.bass as bass
import concourse.mybir as mybir
import concourse.tile as tile
from concourse.bass_utils import run_bass_kernel_spmd
from concourse.masks import make_identity

B, S, D = 4, 2048, 1024
PB = 128
NT = 8  # q-tiles per core
E_SCHED = [2, 16, 4, 14, 6, 12, 8, 10]  # padded key extents per tile (128-blocks)
QBLOCKS = [
    [0, 15, 2, 13, 4, 11, 6, 9],  # parity 0
    [1, 14, 3, 12, 5, 10, 7, 8],  # parity 1
]
SCALE = 1.0 / 32.0  # 1/sqrt(1024)
NEG = -1.0e30
F32 = mybir.dt.float32
F32R = mybir.dt.float32r  # reduced-precision matmul fmt, 4x PE rate at N>=256
KD = D // PB  # 8 contraction blocks


def _chunks(width, step=512):
    return [(c, min(step, width - c)) for c in range(0, width, step)]


def _build_program():
    nc = bacc.Bacc(trn_type="TRN2", target_bir_lowering=False, debug=False, num_swdge_queues=4)

    xT = nc.dram_tensor("xT", [D, S], F32, kind="ExternalInput").ap()
    xTq = nc.dram_tensor("xTq", [D, NT * PB], F32, kind="ExternalInput").ap()
    Wq = nc.dram_tensor("Wq", [D, D], F32, kind="ExternalInput").ap()
    Wk = nc.dram_tensor("Wk", [D, D], F32, kind="ExternalInput").ap()
    Wv = nc.dram_tensor("Wv", [D, D], F32, kind="ExternalInput").ap()
    masks = nc.dram_tensor("masks", [NT, PB, 256], F32, kind="ExternalInput").ap()
    s_out = nc.dram_tensor("s_out", [NT, PB, S], F32, kind="ExternalOutput").ap()
    ctx_out = nc.dram_tensor("ctx_out", [NT, PB, D], F32, kind="ExternalOutput").ap()
    kT_d = nc.dram_tensor("kT_d", [D, S], F32).ap()
    qT_d = nc.dram_tensor("qT_d", [D, NT * PB], F32).ap()

    NQ = NT * PB  # 1024 query rows per core

    with tile.TileContext(nc) as tc, ExitStack() as top:
        const = top.enter_context(tc.tile_pool(name="const", bufs=1))
        vpool = top.enter_context(tc.tile_pool(name="vres", bufs=1))

        ident = const.tile([PB, PB], F32)
        make_identity(nc, ident[:, :])
        mask_sb = const.tile([PB, NT * 256], F32)
        v = vpool.tile([PB, (S // PB) * D], F32R)  # V resident (f32r): block s at cols s*D

        # ---- phase A: projections ----
        with tc.tile_pool(name="xtp", bufs=1) as xtp:
            xt = xtp.tile([PB, KD * S], F32)  # x.T resident: block k at cols k*S

            # A1: Q^T first (needs only xtq, 4MB) so PE starts early while
            # the bigger xt load streams in; then K^T.
            with (
                tc.tile_pool(name="xtqp", bufs=1) as xtqp,
                tc.tile_pool(name="a1w", bufs=2) as a1w,
                tc.tile_pool(name="bnc", bufs=8) as bnc,
                tc.tile_pool(name="psA", bufs=6, space="PSUM") as psA,
            ):
                # first Q weight block loads ahead of the bulk x loads so
                # the PE can start within a few us
                wqt0 = a1w.tile([PB, KD * PB], F32, tag="wq", name="wqt0")
                nc.sync.dma_start(
                    out=wqt0[:, :].rearrange("p (k m) -> p k m", k=KD),
                    in_=Wq.rearrange("(k p) m -> p k m", p=PB)[:, :, 0:PB],
                )
                xtq = xtqp.tile([PB, KD * NQ], F32)
                for k in range(KD):
                    nc.sync.dma_start(
                        out=xtq[:, k * NQ : (k + 1) * NQ],
                        in_=xTq[k * PB : (k + 1) * PB, :],
                    )
                for k in range(KD):
                    nc.sync.dma_start(
                        out=xt[:, k * S : (k + 1) * S], in_=xT[k * PB : (k + 1) * PB, :]
                    )
                nc.sync.dma_start(
                    out=mask_sb[:, :].rearrange("p (t c) -> p t c", t=NT),
                    in_=masks.rearrange("t p c -> p t c"),
                )
                for m in range(KD):
                    if m == 0:
                        wqt = wqt0
                    else:
                        wqt = a1w.tile([PB, KD * PB], F32, tag="wq", name="wqt")
                        nc.sync.dma_start(
                            out=wqt[:, :].rearrange("p (k m) -> p k m", k=KD),
                            in_=Wq.rearrange("(k p) m -> p k m", p=PB)[
                                :, :, m * PB : (m + 1) * PB
                            ],
                        )
                    qchunks = _chunks(NQ)
                    psq = [
                        psA.tile([PB, 512], F32, tag="psA", name=f"psQ{ci}")
                        for ci in range(len(qchunks))
                    ]
                    for k in range(KD):
                        for ci, (c, w) in enumerate(qchunks):
                            nc.tensor.matmul(
                                psq[ci][:, :w],
                                wqt[:, k * PB : (k + 1) * PB],
                                xtq[:, k * NQ + c : k * NQ + c + w],
                                start=(k == 0),
                                stop=(k == KD - 1),
                            )
                    for ci, (c, w) in enumerate(qchunks):
                        ob = bnc.tile([PB, 512], F32, tag="ob")
                        nc.scalar.copy(ob[:, :w], psq[ci][:, :w])
                        nc.gpsimd.dma_start(
                            out=qT_d[m * PB : (m + 1) * PB, c : c + w], in_=ob[:, :w]
                        )
                for m in range(KD):
                    wkt = a1w.tile([PB, KD * PB], F32, tag="wk", name="wkt")
                    nc.sync.dma_start(
                        out=wkt[:, :].rearrange("p (k m) -> p k m", k=KD),
                        in_=Wk.rearrange("(k p) m -> p k m", p=PB)[
                            :, :, m * PB : (m + 1) * PB
                        ],
                    )
                    # k-outer so the first matmuls only need xt block 0
                    # (hides the initial xt DMA under compute)
                    kchunks = _chunks(S)
                    pss = [
                        psA.tile([PB, 512], F32, tag="psA", name=f"psK{ci}")
                        for ci in range(len(kchunks))
                    ]
                    for k in range(KD):
                        for ci, (c, w) in enumerate(kchunks):
                            nc.tensor.matmul(
                                pss[ci][:, :w],
                                wkt[:, k * PB : (k + 1) * PB],
                                xt[:, k * S + c : k * S + c + w],
                                start=(k == 0),
                                stop=(k == KD - 1),
                            )
                    for ci, (c, w) in enumerate(kchunks):
                        ob = bnc.tile([PB, 512], F32, tag="ob")
                        nc.scalar.copy(ob[:, :w], pss[ci][:, :w])
                        nc.gpsimd.dma_start(
                            out=kT_d[m * PB : (m + 1) * PB, c : c + w], in_=ob[:, :w]
                        )

            # A2: V = x @ Wv in float32r (value path tolerates reduced
            # precision; logit path stays fp32), V resident as f32r
            with (
                tc.tile_pool(name="wvp", bufs=1) as wvp,
                tc.tile_pool(name="wvstg", bufs=2) as wvstg,
                tc.tile_pool(name="xtrp", bufs=12) as xtrp,
                tc.tile_pool(name="psV", bufs=4, space="PSUM") as psV,
            ):
                wv = wvp.tile([PB, KD * D], F32R)  # Wv block k at cols k*D
                for k in range(KD):
                    stg = wvstg.tile([PB, D], F32, tag="stg", name="stg")
                    nc.sync.dma_start(
                        out=stg[:, :], in_=Wv[k * PB : (k + 1) * PB, :]
                    )
                    nc.vector.tensor_copy(wv[:, k * D : (k + 1) * D], stg[:, :])
                for sblk in range(S // PB):
                    xtr = []
                    for k in range(KD):
                        xr = xtrp.tile([PB, PB], F32R, tag="xtr", name="xr")
                        nc.vector.tensor_copy(
                            xr[:, :],
                            xt[:, k * S + sblk * PB : k * S + (sblk + 1) * PB],
                        )
                        xtr.append(xr)
                    for h in range(2):
                        ps = psV.tile([PB, 512], F32, tag="psV")
                        for k in range(KD):
                            nc.tensor.matmul(
                                ps[:, :],
                                xtr[k][:, :],
                                wv[:, k * D + h * 512 : k * D + (h + 1) * 512],
                                start=(k == 0),
                                stop=(k == KD - 1),
                            )
                        nc.scalar.copy(
                            v[:, sblk * D + h * 512 : sblk * D + (h + 1) * 512],
                            ps[:, :],
                        )

        # ---- phase B: attention ----
        with (
            tc.tile_pool(name="qtp", bufs=2) as qtp,
            tc.tile_pool(name="ktcp", bufs=9) as ktcp,
            tc.tile_pool(name="pp", bufs=2) as pp,
            tc.tile_pool(name="pnp", bufs=2) as pnp,
            tc.tile_pool(name="ptp", bufs=4) as ptp,
            tc.tile_pool(name="cbp", bufs=4) as cbp,
            tc.tile_pool(name="stat", bufs=3) as stat,
            tc.tile_pool(name="psS", bufs=2, space="PSUM") as psS,
            tc.tile_pool(name="psT", bufs=2, space="PSUM") as psT,
            tc.tile_pool(name="psC", bufs=2, space="PSUM") as psC,
        ):
            HALF = 1024
            for t in range(NT):
                E = E_SCHED[t]
                W = E * PB
                qt = qtp.tile([PB, KD * PB], F32, tag="qt")
                nc.sync.dma_start(
                    out=qt[:, :].rearrange("p (k q) -> p k q", k=KD),
                    in_=qT_d.rearrange("(k p) q -> p k q", p=PB)[
                        :, :, t * PB : (t + 1) * PB
                    ],
                )
                # S psum in half-tiles of 1024 so tile t+1 can reuse slots
                # as soon as tile t's exp drains them
                nhalf = (W + HALF - 1) // HALF
                pshs = [
                    psS.tile([PB, HALF], F32, tag="ps", name=f"psh{h}")
                    for h in range(nhalf)
                ]
                ktcs = []
                for k in range(KD):
                    ktc = ktcp.tile([PB, S], F32, tag="ktc", name="ktc")
                    nc.sync.dma_start(
                        out=ktc[:, :W], in_=kT_d[k * PB : (k + 1) * PB, 0:W]
                    )
                    ktcs.append(ktc)
                for (c, w) in _chunks(W):
                    hi, lo = c // HALF, c % HALF
                    for k in range(KD):
                        nc.tensor.matmul(
                            pshs[hi][:, lo : lo + w],
                            qt[:, k * PB : (k + 1) * PB],
                            ktcs[k][:, c : c + w],
                            start=(k == 0),
                            stop=(k == KD - 1),
                        )
                # causal mask on the last two key blocks (256-aligned, never
                # spans a half boundary)
                mhi, mlo = (W - 256) // HALF, (W - 256) % HALF
                nc.vector.tensor_add(
                    pshs[mhi][:, mlo : mlo + 256],
                    pshs[mhi][:, mlo : mlo + 256],
                    mask_sb[:, t * 256 : (t + 1) * 256],
                )
                hws = [min(HALF, W - h * HALF) for h in range(nhalf)]
                if nhalf == 1:
                    rowmax = stat.tile([PB, 1], F32, tag="rmax")
                    nc.vector.reduce_max(
                        rowmax[:, :], pshs[0][:, : hws[0]], axis=mybir.AxisListType.X
                    )
                else:
                    hmax = stat.tile([PB, 2], F32, tag="hmax")
                    for h in range(nhalf):
                        nc.vector.reduce_max(
                            hmax[:, h : h + 1],
                            pshs[h][:, : hws[h]],
                            axis=mybir.AxisListType.X,
                        )
                    rowmax = stat.tile([PB, 1], F32, tag="rmax")
                    nc.vector.reduce_max(
                        rowmax[:, :], hmax[:, :], axis=mybir.AxisListType.X
                    )
                ebias = stat.tile([PB, 1], F32, tag="ebias")
                nc.vector.tensor_scalar_mul(ebias[:, :], rowmax[:, :], -SCALE)
                p_t = pp.tile([PB, S], F32, tag="p")
                for h in range(nhalf):
                    nc.scalar.activation(
                        p_t[:, h * HALF : h * HALF + hws[h]],
                        pshs[h][:, : hws[h]],
                        mybir.ActivationFunctionType.Exp,
                        bias=ebias[:, :],
                        scale=SCALE,
                    )
                rowsum = stat.tile([PB, 1], F32, tag="rsum")
                nc.vector.reduce_sum(rowsum[:, :], p_t[:, :W], axis=mybir.AxisListType.X)
                recip = stat.tile([PB, 1], F32, tag="recip")
                nc.vector.reciprocal(recip[:, :], rowsum[:, :])
                pn = pnp.tile([PB, S], F32, tag="pn")
                nc.vector.tensor_scalar_mul(pn[:, :W], p_t[:, :W], recip[:, :])
                nc.gpsimd.dma_start(out=s_out[t, :, 0:W], in_=pn[:, :W])
                # ctx = P^T.T @ V, scaled by recip
                pc = [
                    psC.tile([PB, 512], F32, tag="pc", name=f"pc{d}")
                    for d in range(2)
                ]
                for e in range(E):
                    pst = psT.tile([PB, PB], F32, tag="pst")
                    nc.tensor.transpose(
                        pst[:, :], p_t[:, e * PB : (e + 1) * PB], ident[:, :]
                    )
                    pte = ptp.tile([PB, PB], F32R, tag="pte")
                    nc.vector.tensor_copy(pte[:, :], pst[:, :])
                    for d in range(2):
                        nc.tensor.matmul(
                            pc[d][:, :],
                            pte[:, :],
                            v[:, e * D + d * 512 : e * D + (d + 1) * 512],
                            start=(e == 0),
                            stop=(e == E - 1),
                        )
                for d in range(2):
                    cb = cbp.tile([PB, 512], F32, tag="cb")
                    nc.scalar.activation(
                        cb[:, :],
                        pc[d][:, :],
                        mybir.ActivationFunctionType.Copy,
                        scale=recip[:, :],
                    )
                    nc.gpsimd.dma_start(
                        out=ctx_out[t, :, d * 512 : (d + 1) * 512], in_=cb[:, :]
                    )
    nc.compile()
    return nc


_CACHE = {}


def _program():
    if "nc" not in _CACHE:
        _CACHE["nc"] = _build_program()
    return _CACHE["nc"]


def _tri_mask():
    r = np.arange(PB)
    return np.where(r[None, :] <= r[:, None], 0.0, NEG).astype(np.float32)


def _masks_for_parity(p):
    tri = _tri_mask()
    m = np.zeros((NT, PB, 256), np.float32)
    for t in range(NT):
        E = E_SCHED[t]
        e = QBLOCKS[p][t] + 1  # true causal extent in blocks
        if e == E:
            m[t, :, 128:] = tri
        elif e == E - 1:
            m[t, :, :128] = tri
            m[t, :, 128:] = NEG
        else:
            raise AssertionError("bad schedule")
    return m


def _in_maps(x, Wq, Wk, Wv):
    maps = []
    pmasks = [_masks_for_parity(0), _masks_for_parity(1)]
    for core in range(8):
        b, p = divmod(core, 2)
        xTb = np.ascontiguousarray(x[b].T)
        qrows = np.concatenate(
            [np.arange(g * PB, (g + 1) * PB) for g in QBLOCKS[p]]
        )
        xTqb = np.ascontiguousarray(xTb[:, qrows])
        maps.append(
            {
                "xT": xTb,
                "xTq": xTqb,
                "Wq": Wq,
                "Wk": Wk,
                "Wv": Wv,
                "masks": pmasks[p],
            }
        )
    return maps


def _assemble(results):
    scores = np.zeros((B, S, S), np.float32)
    ctx = np.empty((B, S, D), np.float32)
    for core in range(8):
        b, p = divmod(core, 2)
        so = results[core]["s_out"]
        co = results[core]["ctx_out"]
        for t, g in enumerate(QBLOCKS[p]):
            scores[b, g * PB : (g + 1) * PB, :] = so[t]
            ctx[b, g * PB : (g + 1) * PB, :] = co[t]
    return ctx, scores


def run_on_cores(x, W_query, W_key, W_value, **spmd_kwargs):
    """Run the 8-core SPMD program; returns (BassKernelResults, (ctx, scores))."""
    x = np.ascontiguousarray(np.asarray(x, dtype=np.float32))
    Wq = np.ascontiguousarray(np.asarray(W_query, dtype=np.float32))
    Wk = np.ascontiguousarray(np.asarray(W_key, dtype=np.float32))
    Wv = np.ascontiguousarray(np.asarray(W_value, dtype=np.float32))
    nc = _program()
    res = run_bass_kernel_spmd(nc, _in_maps(x, Wq, Wk, Wv), list(range(8)), **spmd_kwargs)
    return res, _assemble(res.results)


def kernel(x, W_query, W_key, W_value):
    _, out = run_on_cores(x, W_query, W_key, W_value)
    return out


# revision 21
# speedup vs baseline: 1.2496x; 1.0426x over previous
"""Causal attention kernel for Trainium2 (8 NeuronCores, Bass/Tile).

Problem: B=4, S=2048, D=1024 fp32.
  qkv = x @ [Wq|Wk|Wv]; S = mask(q@k.T); P = softmax(S/sqrt(D)); ctx = P@v
  returns (context [4,2048,1024], attn_scores [4,2048,2048])

Sharding: core = (batch b = core//2, parity p = core%2). Each core handles
8 query blocks of 128 rows of its batch, chosen causally balanced:
  parity 0 -> q-blocks [0,15,2,13,4,11,6,9]
  parity 1 -> q-blocks [1,14,3,12,5,10,7,8]
Both parities run the same SPMD program with padded key extents
E_SCHED = [2,16,4,14,6,12,8,10] (in 128-blocks); the true causal boundary
(which is parity-dependent) is applied via a host-provided additive mask
over the last 256 key columns of each tile.

On-device phases (single NEFF):
  A1: K^T = (Wk.T @ x.T) and Q^T (for the core's q rows) -> DRAM bounce
  A2: V = x @ Wv -> resident in SBUF
  B:  per q-tile: S = Q^T.T @ K^T (psum), mask, row-max, exp((S-max)/32),
      P -> DMA out (normalized), P^T via PE transpose, ctx = P^T.T @ V,
      scale by 1/rowsum -> DMA out.

All matmuls fp32 (logits need fp32 accuracy: softmax here is near-one-hot
with min top-2 logit gap ~0.57).
"""

import os
import sys
from contextlib import ExitStack

import numpy as np

import concourse.bacc as bacc
import concourse.bass as bass
import concourse.mybir as mybir
import concourse.tile as tile
from concourse.bass_utils import run_bass_kernel_spmd
from concourse.masks import make_identity

B, S, D = 4, 2048, 1024
PB = 128
NT = 8  # q-tiles per core
E_SCHED = [2, 16, 4, 14, 6, 12, 8, 10]  # padded key extents per tile (128-blocks)
QBLOCKS = [
    [0, 15, 2, 13, 4, 11, 6, 9],  # parity 0
    [1, 14, 3, 12, 5, 10, 7, 8],  # parity 1
]
SCALE = 1.0 / 32.0  # 1/sqrt(1024)
NEG = -1.0e30
F32 = mybir.dt.float32
F32R = mybir.dt.float32r  # reduced-precision matmul fmt, 4x PE rate at N>=256
KD = D // PB  # 8 contraction blocks


def _chunks(width, step=512):
    return [(c, min(step, width - c)) for c in range(0, width, step)]


def _build_program():
    nc = bacc.Bacc(trn_type="TRN2", target_bir_lowering=False, debug=False, num_swdge_queues=4)

    xT = nc.dram_tensor("xT", [D, S], F32, kind="ExternalInput").ap()
    xTq = nc.dram_tensor("xTq", [D, NT * PB], F32, kind="ExternalInput").ap()
    Wq = nc.dram_tensor("Wq", [D, D], F32, kind="ExternalInput").ap()
    Wk = nc.dram_tensor("Wk", [D, D], F32, kind="ExternalInput").ap()
    Wv = nc.dram_tensor("Wv", [D, D], F32, kind="ExternalInput").ap()
    masks = nc.dram_tensor("masks", [NT, PB, 256], F32, kind="ExternalInput").ap()
    s_out = nc.dram_tensor("s_out", [NT, PB, S], F32, kind="ExternalOutput").ap()
    ctx_out = nc.dram_tensor("ctx_out", [NT, PB, D], F32, kind="ExternalOutput").ap()
    kT_d = nc.dram_tensor("kT_d", [D, S], F32).ap()
    qT_d = nc.dram_tensor("qT_d", [D, NT * PB], F32).ap()

    NQ = NT * PB  # 1024 query rows per core

    with tile.TileContext(nc) as tc, ExitStack() as top:
        const = top.enter_context(tc.tile_pool(name="const", bufs=1))
        vpool = top.enter_context(tc.tile_pool(name="vres", bufs=1))

        ident = const.tile([PB, PB], F32)
        make_identity(nc, ident[:, :])
        mask_sb = const.tile([PB, NT * 256], F32)
        v = vpool.tile([PB, (S // PB) * D], F32R)  # V resident (f32r): block s at cols s*D

        # ---- phase A: projections ----
        with tc.tile_pool(name="xtp", bufs=1) as xtp:
            xt = xtp.tile([PB, KD * S], F32)  # x.T resident: block k at cols k*S

            # A1: Q^T first (needs only xtq, 4MB) so PE starts early while
            # the bigger xt load streams in; then K^T.
            with (
                tc.tile_pool(name="xtqp", bufs=1) as xtqp,
                tc.tile_pool(name="a1w", bufs=2) as a1w,
                tc.tile_pool(name="bnc", bufs=8) as bnc,
                tc.tile_pool(name="psA", bufs=6, space="PSUM") as psA,
            ):
                # first Q weight block loads ahead of the bulk x loads so
                # the PE can start within a few us
                wqt0 = a1w.tile([PB, KD * PB], F32, tag="wq", name="wqt0")
                nc.gpsimd.dma_start(
                    out=wqt0[:, :].rearrange("p (k m) -> p k m", k=KD),
                    in_=Wq.rearrange("(k p) m -> p k m", p=PB)[:, :, 0:PB],
                )
                xtq = xtqp.tile([PB, KD * NQ], F32)
                for k in range(KD):
                    nc.sync.dma_start(
                        out=xtq[:, k * NQ : (k + 1) * NQ],
                        in_=xTq[k * PB : (k + 1) * PB, :],
                    )
                for k in range(KD):
                    nc.sync.dma_start(
                        out=xt[:, k * S : (k + 1) * S], in_=xT[k * PB : (k + 1) * PB, :]
                    )
                nc.sync.dma_start(
                    out=mask_sb[:, :].rearrange("p (t c) -> p t c", t=NT),
                    in_=masks.rearrange("t p c -> p t c"),
                )
                for m in range(KD):
                    if m == 0:
                        wqt = wqt0
                    else:
                        wqt = a1w.tile([PB, KD * PB], F32, tag="wq", name="wqt")
                        nc.gpsimd.dma_start(
                            out=wqt[:, :].rearrange("p (k m) -> p k m", k=KD),
                            in_=Wq.rearrange("(k p) m -> p k m", p=PB)[
                                :, :, m * PB : (m + 1) * PB
                            ],
                        )
                    qchunks = _chunks(NQ)
                    psq = [
                        psA.tile([PB, 512], F32, tag="psA", name=f"psQ{ci}")
                        for ci in range(len(qchunks))
                    ]
                    for k in range(KD):
                        for ci, (c, w) in enumerate(qchunks):
                            nc.tensor.matmul(
                                psq[ci][:, :w],
                                wqt[:, k * PB : (k + 1) * PB],
                                xtq[:, k * NQ + c : k * NQ + c + w],
                                start=(k == 0),
                                stop=(k == KD - 1),
                            )
                    for ci, (c, w) in enumerate(qchunks):
                        ob = bnc.tile([PB, 512], F32, tag="ob")
                        nc.scalar.copy(ob[:, :w], psq[ci][:, :w])
                        nc.gpsimd.dma_start(
                            out=qT_d[m * PB : (m + 1) * PB, c : c + w], in_=ob[:, :w]
                        )
                for m in range(KD):
                    wkt = a1w.tile([PB, KD * PB], F32, tag="wk", name="wkt")
                    nc.gpsimd.dma_start(
                        out=wkt[:, :].rearrange("p (k m) -> p k m", k=KD),
                        in_=Wk.rearrange("(k p) m -> p k m", p=PB)[
                            :, :, m * PB : (m + 1) * PB
                        ],
                    )
                    # k-outer so the first matmuls only need xt block 0
                    # (hides the initial xt DMA under compute)
                    kchunks = _chunks(S)
                    pss = [
                        psA.tile([PB, 512], F32, tag="psA", name=f"psK{ci}")
                        for ci in range(len(kchunks))
                    ]
                    for k in range(KD):
                        for ci, (c, w) in enumerate(kchunks):
                            nc.tensor.matmul(
                                pss[ci][:, :w],
                                wkt[:, k * PB : (k + 1) * PB],
                                xt[:, k * S + c : k * S + c + w],
                                start=(k == 0),
                                stop=(k == KD - 1),
                            )
                    for ci, (c, w) in enumerate(kchunks):
                        ob = bnc.tile([PB, 512], F32, tag="ob")
                        nc.scalar.copy(ob[:, :w], pss[ci][:, :w])
                        nc.gpsimd.dma_start(
                            out=kT_d[m * PB : (m + 1) * PB, c : c + w], in_=ob[:, :w]
                        )

            # A2: V = x @ Wv in float32r (value path tolerates reduced
            # precision; logit path stays fp32), V resident as f32r
            with (
                tc.tile_pool(name="wvp", bufs=1) as wvp,
                tc.tile_pool(name="wvstg", bufs=2) as wvstg,
                tc.tile_pool(name="xtrp", bufs=12) as xtrp,
                tc.tile_pool(name="psV", bufs=4, space="PSUM") as psV,
            ):
                wv = wvp.tile([PB, KD * D], F32R)  # Wv block k at cols k*D
                for k in range(KD):
                    stg = wvstg.tile([PB, D], F32, tag="stg", name="stg")
                    nc.sync.dma_start(
                        out=stg[:, :], in_=Wv[k * PB : (k + 1) * PB, :]
                    )
                    nc.vector.tensor_copy(wv[:, k * D : (k + 1) * D], stg[:, :])
                for sblk in range(S // PB):
                    xtr = []
                    for k in range(KD):
                        xr = xtrp.tile([PB, PB], F32R, tag="xtr", name="xr")
                        nc.vector.tensor_copy(
                            xr[:, :],
                            xt[:, k * S + sblk * PB : k * S + (sblk + 1) * PB],
                        )
                        xtr.append(xr)
                    for h in range(2):
                        ps = psV.tile([PB, 512], F32, tag="psV")
                        for k in range(KD):
                            nc.tensor.matmul(
                                ps[:, :],
                                xtr[k][:, :],
                                wv[:, k * D + h * 512 : k * D + (h + 1) * 512],
                                start=(k == 0),
                                stop=(k == KD - 1),
                            )
                        nc.scalar.copy(
                            v[:, sblk * D + h * 512 : sblk * D + (h + 1) * 512],
                            ps[:, :],
                        )

        # ---- phase B: attention ----
        with (
            tc.tile_pool(name="qtp", bufs=2) as qtp,
            tc.tile_pool(name="ktcp", bufs=2) as ktcp,
            tc.tile_pool(name="pp", bufs=2) as pp,
            tc.tile_pool(name="pnp", bufs=2) as pnp,
            tc.tile_pool(name="ptp", bufs=4) as ptp,
            tc.tile_pool(name="cbp", bufs=4) as cbp,
            tc.tile_pool(name="stat", bufs=3) as stat,
            tc.tile_pool(name="psS", bufs=2, space="PSUM") as psS,
            tc.tile_pool(name="psT", bufs=2, space="PSUM") as psT,
            tc.tile_pool(name="psC", bufs=2, space="PSUM") as psC,
        ):
            HALF = 1024
            for t in sorted(range(NT), key=lambda tt: -E_SCHED[tt]):
                E = E_SCHED[t]
                W = E * PB
                qt = qtp.tile([PB, KD * PB], F32, tag="qt")
                nc.sync.dma_start(
                    out=qt[:, :].rearrange("p (k q) -> p k q", k=KD),
                    in_=qT_d.rearrange("(k p) q -> p k q", p=PB)[
                        :, :, t * PB : (t + 1) * PB
                    ],
                )
                # S psum in half-tiles of 1024 so tile t+1 can reuse slots
                # as soon as tile t's exp drains them
                nhalf = (W + HALF - 1) // HALF
                pshs = [
                    psS.tile([PB, HALF], F32, tag="ps", name=f"psh{h}")
                    for h in range(nhalf)
                ]
                ktcs = []
                for h in range(nhalf):
                    hw = min(HALF, W - h * HALF)
                    ktc = ktcp.tile([PB, KD * HALF], F32, tag="ktc", name="ktc")
                    nc.sync.dma_start(
                        out=ktc[:, :].rearrange("p (k s) -> p k s", k=KD)[:, :, :hw],
                        in_=kT_d.rearrange("(k p) s -> p k s", p=PB)[
                            :, :, h * HALF : h * HALF + hw
                        ],
                    )
                    ktcs.append(ktc)
                for (c, w) in _chunks(W):
                    hi, lo = c // HALF, c % HALF
                    for k in range(KD):
                        nc.tensor.matmul(
                            pshs[hi][:, lo : lo + w],
                            qt[:, k * PB : (k + 1) * PB],
                            ktcs[hi][:, k * HALF + lo : k * HALF + lo + w],
                            start=(k == 0),
                            stop=(k == KD - 1),
                        )
                # causal mask on the last two key blocks (256-aligned, never
                # spans a half boundary)
                mhi, mlo = (W - 256) // HALF, (W - 256) % HALF
                nc.vector.tensor_add(
                    pshs[mhi][:, mlo : mlo + 256],
                    pshs[mhi][:, mlo : mlo + 256],
                    mask_sb[:, t * 256 : (t + 1) * 256],
                )
                hws = [min(HALF, W - h * HALF) for h in range(nhalf)]
                if nhalf == 1:
                    rowmax = stat.tile([PB, 1], F32, tag="rmax")
                    nc.vector.reduce_max(
                        rowmax[:, :], pshs[0][:, : hws[0]], axis=mybir.AxisListType.X
                    )
                else:
                    hmax = stat.tile([PB, 2], F32, tag="hmax")
                    for h in range(nhalf):
                        nc.vector.reduce_max(
                            hmax[:, h : h + 1],
                            pshs[h][:, : hws[h]],
                            axis=mybir.AxisListType.X,
                        )
                    rowmax = stat.tile([PB, 1], F32, tag="rmax")
                    nc.vector.reduce_max(
                        rowmax[:, :], hmax[:, :], axis=mybir.AxisListType.X
                    )
                ebias = stat.tile([PB, 1], F32, tag="ebias")
                nc.vector.tensor_scalar_mul(ebias[:, :], rowmax[:, :], -SCALE)
                p_t = pp.tile([PB, S], F32, tag="p")
                for h in range(nhalf):
                    nc.scalar.activation(
                        p_t[:, h * HALF : h * HALF + hws[h]],
                        pshs[h][:, : hws[h]],
                        mybir.ActivationFunctionType.Exp,
                        bias=ebias[:, :],
                        scale=SCALE,
                    )
                rowsum = stat.tile([PB, 1], F32, tag="rsum")
                nc.vector.reduce_sum(rowsum[:, :], p_t[:, :W], axis=mybir.AxisListType.X)
                recip = stat.tile([PB, 1], F32, tag="recip")
                nc.vector.reciprocal(recip[:, :], rowsum[:, :])
                pn = pnp.tile([PB, S], F32, tag="pn")
                nc.vector.tensor_scalar_mul(pn[:, :W], p_t[:, :W], recip[:, :])
                nc.gpsimd.dma_start(out=s_out[t, :, 0:W], in_=pn[:, :W])
                # ctx = P^T.T @ V, scaled by recip
                pc = [
                    psC.tile([PB, 512], F32, tag="pc", name=f"pc{d}")
                    for d in range(2)
                ]
                for e in range(E):
                    pst = psT.tile([PB, PB], F32, tag="pst")
                    nc.tensor.transpose(
                        pst[:, :], p_t[:, e * PB : (e + 1) * PB], ident[:, :]
                    )
                    pte = ptp.tile([PB, PB], F32R, tag="pte")
                    nc.vector.tensor_copy(pte[:, :], pst[:, :])
                    for d in range(2):
                        nc.tensor.matmul(
                            pc[d][:, :],
                            pte[:, :],
                            v[:, e * D + d * 512 : e * D + (d + 1) * 512],
                            start=(e == 0),
                            stop=(e == E - 1),
                        )
                for d in range(2):
                    cb = cbp.tile([PB, 512], F32, tag="cb")
                    nc.scalar.activation(
                        cb[:, :],
                        pc[d][:, :],
                        mybir.ActivationFunctionType.Copy,
                        scale=recip[:, :],
                    )
                    nc.gpsimd.dma_start(
                        out=ctx_out[t, :, d * 512 : (d + 1) * 512], in_=cb[:, :]
                    )
    nc.compile()
    return nc


_CACHE = {}


def _program():
    if "nc" not in _CACHE:
        _CACHE["nc"] = _build_program()
    return _CACHE["nc"]


def _tri_mask():
    r = np.arange(PB)
    return np.where(r[None, :] <= r[:, None], 0.0, NEG).astype(np.float32)


def _masks_for_parity(p):
    tri = _tri_mask()
    m = np.zeros((NT, PB, 256), np.float32)
    for t in range(NT):
        E = E_SCHED[t]
        e = QBLOCKS[p][t] + 1  # true causal extent in blocks
        if e == E:
            m[t, :, 128:] = tri
        elif e == E - 1:
            m[t, :, :128] = tri
            m[t, :, 128:] = NEG
        else:
            raise AssertionError("bad schedule")
    return m


def _in_maps(x, Wq, Wk, Wv):
    maps = []
    pmasks = [_masks_for_parity(0), _masks_for_parity(1)]
    for core in range(8):
        b, p = divmod(core, 2)
        xTb = np.ascontiguousarray(x[b].T)
        qrows = np.concatenate(
            [np.arange(g * PB, (g + 1) * PB) for g in QBLOCKS[p]]
        )
        xTqb = np.ascontiguousarray(xTb[:, qrows])
        maps.append(
            {
                "xT": xTb,
                "xTq": xTqb,
                "Wq": Wq,
                "Wk": Wk,
                "Wv": Wv,
                "masks": pmasks[p],
            }
        )
    return maps


def _assemble(results):
    scores = np.zeros((B, S, S), np.float32)
    ctx = np.empty((B, S, D), np.float32)
    for core in range(8):
        b, p = divmod(core, 2)
        so = results[core]["s_out"]
        co = results[core]["ctx_out"]
        for t, g in enumerate(QBLOCKS[p]):
            scores[b, g * PB : (g + 1) * PB, :] = so[t]
            ctx[b, g * PB : (g + 1) * PB, :] = co[t]
    return ctx, scores


def run_on_cores(x, W_query, W_key, W_value, **spmd_kwargs):
    """Run the 8-core SPMD program; returns (BassKernelResults, (ctx, scores))."""
    x = np.ascontiguousarray(np.asarray(x, dtype=np.float32))
    Wq = np.ascontiguousarray(np.asarray(W_query, dtype=np.float32))
    Wk = np.ascontiguousarray(np.asarray(W_key, dtype=np.float32))
    Wv = np.ascontiguousarray(np.asarray(W_value, dtype=np.float32))
    nc = _program()
    res = run_bass_kernel_spmd(nc, _in_maps(x, Wq, Wk, Wv), list(range(8)), **spmd_kwargs)
    return res, _assemble(res.results)


def kernel(x, W_query, W_key, W_value):
    _, out = run_on_cores(x, W_query, W_key, W_value)
    return out
